# revision 41
# baseline (speedup 1.0000x reference)
"""Trainium2 Bass kernel: single-layer causal attention block (q/k/v/o + RoPE).

Sharding: 8 cores = 2 batches x 4 head-groups (4 heads each).
Per core (SPMD, differs only in input data), all matmul operands bf16:
  - q/k projections emit a merged per-head [even(32); odd(32)] row layout so
    each score tile is ONE 64-contraction matmul (vs 2x32 split).
  - RoPE: partner rows fetched via a PE permutation matmul (row r <- r^32),
    then dst = A*CS + rot*SN in 3 full-width DVE ops per 512-chunk.
  - scores^T = K-stationary matmul -> exp (bf16) on ACT -> multiplicative
    causal mask on diagonal tiles -> PV with a ones-column rowsum.
  - softmax normalization batched: rowsums -> one reciprocal_approx_fast,
    per-(head,chunk) broadcast via one-hot-stationary PE matmul.
  - o_proj partial [2048, 1024] emitted in bf16.
Host: sums the 4 per-head-group partials per batch (row-sharded o_proj
unshard) in f32 and stacks the 2 batches.
"""

import os
import sys

import numpy as np

sys.path.insert(0, "/opt/trn_rl_repo")

import concourse.bass as bass  # noqa: E402
import concourse.tile as tile  # noqa: E402
from concourse import bacc, mybir  # noqa: E402
from concourse import bass_utils  # noqa: E402

B, S, D, H, DK = 2, 2048, 1024, 16, 64
NCORES = 8
HPC = H // 4  # 4 heads per core
CW = HPC * DK  # 256 head-dim columns per core
VW = DK + 1  # 65: v width per head incl ones column
ND = D // 128  # 8 contraction chunks
NS = S // 128  # 16 s-tiles
NSC = S // 512  # 4 s-chunks
ROPE_THETA = 10000.0

F32 = mybir.dt.float32
BF16 = mybir.dt.bfloat16
EXP = mybir.ActivationFunctionType.Exp


def _build_kernel(tc, nc, xt, wq, wk, wv, wo, cs, sn, mk, pm, ehm, out):
    from contextlib import ExitStack
    _stack = ExitStack()
    constp = _stack.enter_context(tc.tile_pool(name="const", bufs=1))
    pers = _stack.enter_context(tc.tile_pool(name="persist", bufs=1))
    xtp = _stack.enter_context(tc.tile_pool(name="xt", bufs=1))

    wq_sb = constp.tile([128, ND * CW], BF16)
    wk_sb = constp.tile([128, ND * CW], BF16)
    wv_sb = constp.tile([128, ND * CW], BF16)
    wo_sb = constp.tile([128, 2 * D], BF16)
    cs_sb = constp.tile([128, S], BF16)
    sn_sb = constp.tile([128, S], BF16)
    mk_sb = constp.tile([128, 1024], BF16)
    pm_sb = constp.tile([128, 128], BF16)
    ehm_sb = constp.tile([128, HPC * 128], BF16)
    xts = xtp.tile([128, ND * S], BF16)
    # DMA order tuned for time-to-first-matmul: weights and the first half of
    # x^T land first; attention/o_proj-only tables go last. Weight tensors are
    # single DMA instructions (rearranged APs) to cut sync-queue issue time.
    xts3 = xts[:].rearrange("p (d s) -> p d s", s=S)
    wq3 = wq[:].rearrange("(d p) c -> p d c", p=128)
    wk3 = wk[:].rearrange("(d p) c -> p d c", p=128)
    wq_sb3 = wq_sb[:].rearrange("p (d c) -> p d c", c=CW)
    wk_sb3 = wk_sb[:].rearrange("p (d c) -> p d c", c=CW)
    # two hwdge queues (sync + scalar) stream the prologue in parallel
    xt3 = xt[:].rearrange("(d p) s -> p d s", p=128)
    nc.sync.dma_start(wq_sb3[:, 0:4], wq3[:, 0:4])
    nc.scalar.dma_start(wq_sb3[:, 4:8], wq3[:, 4:8])
    nc.sync.dma_start(xts3[:, 0:4, 0:512], xt3[:, 0:4, 0:512])
    nc.scalar.dma_start(xts3[:, 4:8, 0:512], xt3[:, 4:8, 0:512])
    nc.sync.dma_start(xts3[:, 0:4, 512:1024], xt3[:, 0:4, 512:1024])
    nc.scalar.dma_start(xts3[:, 4:8, 512:1024], xt3[:, 4:8, 512:1024])
    nc.sync.dma_start(wk_sb3[:, 0:4], wk3[:, 0:4])
    nc.scalar.dma_start(wk_sb3[:, 4:8], wk3[:, 4:8])
    nc.sync.dma_start(cs_sb[:], cs[:])
    nc.scalar.dma_start(sn_sb[:], sn[:])
    nc.sync.dma_start(pm_sb[:], pm[:])
    nc.scalar.dma_start(wv_sb[:].rearrange("p (d c) -> p d c", c=CW),
                        wv[:].rearrange("(d p) c -> p d c", p=128))
    nc.sync.dma_start(xts3[:, 0:4, 1024:S], xt3[:, 0:4, 1024:S])
    nc.scalar.dma_start(xts3[:, 4:8, 1024:S], xt3[:, 4:8, 1024:S])
    nc.sync.dma_start(mk_sb[:], mk[:])
    nc.scalar.dma_start(ehm_sb[:], ehm[:])
    nc.sync.dma_start(wo_sb[:].rearrange("p (b c) -> p b c", c=D),
                      wo[:].rearrange("(b p) c -> p b c", p=128))

    # q/k: one [64, S] tile per head, rows = [even(32); odd(32)], so every
    # score matmul runs at tile_position (0,0) (no PE retile between score
    # and PV matmuls).
    qh = [pers.tile([64, S], BF16, name=f"qh{_h}") for _h in range(HPC)]
    kh = [pers.tile([64, S], BF16, name=f"kh{_h}") for _h in range(HPC)]
    v_sb = pers.tile([128, NS * HPC * VW], BF16)
    ctx_sb = pers.tile([128, 2 * S], BF16)
    # rowsum rows live at partition h*32 (engine partition starts must be
    # multiples of 32); unused rows memset to 1.0 so reciprocal stays finite
    sums_sb = pers.tile([128, S], F32)
    rcp_sb = pers.tile([128, S], F32)
    rcpb_sb = pers.tile([128, S], BF16)

    v3 = v_sb[:].rearrange("p (t c) -> p t c", c=VW)
    nc.vector.memset(v3[:, :, DK:DK + 1], 1.0)
    nc.vector.memset(sums_sb[:], 1.0)

    # ---- Phase 1: projections + RoPE ----
    # Per sc-chunk: a single-bank accumulation chain over d (consecutive PE
    # matmuls hit the same PSUM bank - the fast PE issue path), then RoPE.
    with tc.tile_pool(name="pp", bufs=4, space="PSUM") as pp, \
         tc.tile_pool(name="rp", bufs=2, space="PSUM") as rp, \
         tc.tile_pool(name="vps", bufs=2, space="PSUM") as vps, \
         tc.tile_pool(name="rsb", bufs=8) as rsb:
        blocks = [(qh[0], qh[1], wq_sb, 0), (qh[2], qh[3], wq_sb, 1),
                  (kh[0], kh[1], wk_sb, 0), (kh[2], kh[3], wk_sb, 1)]
        for dst0, dst1, wsb, blk in blocks:
            for sc in range(NSC):
                pst = pp.tile([128, 512], F32, tag="pjt")
                for d in range(ND):
                    nc.tensor.matmul(
                        pst[:],
                        wsb[:, d * CW + blk * 128: d * CW + blk * 128 + 128],
                        xts[:, d * S + sc * 512: d * S + (sc + 1) * 512],
                        start=(d == 0), stop=(d == ND - 1))
                araw = rsb.tile([128, 512], BF16, tag="araw")
                nc.vector.tensor_copy(araw[:], pst[:])
                prot = rp.tile([128, 512], F32)
                nc.tensor.matmul(prot[:], pm_sb[:], araw[:], start=True, stop=True)
                rot = rsb.tile([128, 512], BF16, tag="rot")
                nc.scalar.copy(rot[:], prot[:])
                t1 = rsb.tile([128, 512], BF16, tag="t1")
                t2 = rsb.tile([128, 512], BF16, tag="t2")
                nc.vector.tensor_mul(t1[:], araw[:], cs_sb[:, sc * 512:(sc + 1) * 512])
                nc.vector.tensor_mul(t2[:], rot[:], sn_sb[:, sc * 512:(sc + 1) * 512])
                nc.vector.tensor_add(dst0[:, sc * 512:(sc + 1) * 512],
                                     t1[0:64, :], t2[0:64, :])
                nc.vector.tensor_add(dst1[:, sc * 512:(sc + 1) * 512],
                                     t1[64:128, :], t2[64:128, :])
        for sm in range(NS):
            pv = vps.tile([128, CW], F32)
            for d in range(ND):
                nc.tensor.matmul(
                    pv[:],
                    xts[:, d * S + sm * 128: d * S + sm * 128 + 128],
                    wv_sb[:, d * CW:(d + 1) * CW],
                    start=(d == 0), stop=(d == ND - 1))
            base = sm * HPC * VW
            dst3 = v_sb[:, base:base + HPC * VW].rearrange("p (h c) -> p h c", c=VW)
            nc.vector.tensor_copy(dst3[:, :, 0:DK],
                                  pv[:].rearrange("p (h c) -> p h c", c=DK))

    # ---- Phase 2: attention + per-chunk normalization + o_proj ----
    # Heads (2hp, 2hp+1) share one [128,1024] psum pair-tile per k-tile: two
    # score matmuls, ONE exp over the pair, two PV accumulations. After both
    # head-pairs finish a q-chunk, normalize it (gpsimd broadcast of the
    # batched reciprocal) and emit its o_proj s-tiles, overlapping the next
    # chunk's attention.
    def o_proj(c, opsp, obp):
        for sm in range(4 * c, 4 * c + 4):
            pos = [opsp.tile([128, 512], F32, name=f"po{_i}", tag=f"po{_i}")
                   for _i in range(2)]
            for do_ in range(2):
                for cb in range(2):
                    nc.tensor.matmul(
                        pos[do_][:],
                        ctx_sb[:, cb * S + sm * 128: cb * S + sm * 128 + 128],
                        wo_sb[:, cb * D + do_ * 512: cb * D + (do_ + 1) * 512],
                        start=(cb == 0), stop=(cb == 1),
                        skip_group_check=True)
            ot = obp.tile([128, 1024], BF16)
            nc.vector.tensor_copy(ot[:, 0:512], pos[0][:])
            nc.scalar.copy(ot[:, 512:1024], pos[1][:])
            nc.sync.dma_start(out[sm * 128:(sm + 1) * 128, :], ot[:])

    with tc.tile_pool(name="sps", bufs=2, space="PSUM") as sps, \
         tc.tile_pool(name="cps", bufs=1, space="PSUM") as cps, \
         tc.tile_pool(name="ops", bufs=1, space="PSUM") as opsp, \
         tc.tile_pool(name="expool", bufs=10) as exp_pool, \
         tc.tile_pool(name="nsb", bufs=4) as nsb, \
         tc.tile_pool(name="obuf", bufs=2) as obp:
        prev_c = None
        for c in (3, 2, 1, 0):
            nsk = 4 * (c + 1)
            for hp in range(2):
                h0, h1 = 2 * hp, 2 * hp + 1
                pctx0 = cps.tile([VW, 512], F32, tag="pctx0")
                pctx1 = cps.tile([VW, 512], F32, tag="pctx1")
                exps = []

                def pv_pair(ta, tb, nsk=nsk, pctx0=pctx0, pctx1=pctx1,
                            exps=exps, h0=h0, h1=h1, c=c):
                    # two consecutive PV accumulations per pctx bank
                    # back-to-back: same-bank chains issue fastest on the PE
                    for s, (hh, pc) in enumerate(((h0, pctx0), (h1, pctx1))):
                        for t in (ta, tb):
                            qo = max(0, t - 4 * c) * 128
                            vbase = t * HPC * VW + hh * VW
                            nc.tensor.matmul(
                                pc[:, qo:512],
                                v_sb[:, vbase:vbase + VW],
                                exps[t][:, s * 512: s * 512 + 512 - qo],
                                start=(t == 0), stop=(t == nsk - 1),
                                skip_group_check=True)

                for pi in range(nsk // 2):
                    for t in (2 * pi, 2 * pi + 1):
                        j = t - 4 * c
                        qo = max(0, j) * 128
                        w = 512 - qo
                        pscore = sps.tile([128, 1024], F32)
                        for s, hh in enumerate((h0, h1)):
                            nc.tensor.matmul(
                                pscore[:, s * 512: s * 512 + w],
                                kh[hh][:, t * 128:(t + 1) * 128],
                                qh[hh][:, c * 512 + qo:(c + 1) * 512],
                                start=True, stop=True,
                                skip_group_check=True)
                        et = exp_pool.tile([128, 1024], BF16)
                        if j <= 0:
                            nc.scalar.activation(et[:], pscore[:], EXP,
                                                 scale=0.125)
                            if j == 0:
                                nc.vector.tensor_mul(et[:], et[:], mk_sb[:])
                        else:
                            # shifted coords: every diagonal mask is the same
                            # triangle (f' >= p), truncated to width w
                            for s in range(2):
                                nc.scalar.activation(
                                    et[:, s * 512:s * 512 + w],
                                    pscore[:, s * 512:s * 512 + w],
                                    EXP, scale=0.125)
                                nc.vector.tensor_mul(
                                    et[:, s * 512:s * 512 + w],
                                    et[:, s * 512:s * 512 + w],
                                    mk_sb[:, 0:w])
                        exps.append(et)
                    if pi >= 3:
                        pv_pair(2 * (pi - 3), 2 * pi - 5)
                for pi in range(max(0, nsk // 2 - 3), nsk // 2):
                    pv_pair(2 * pi, 2 * pi + 1)
                # ctx on DVE, rowsums on ACT: both streams drain in parallel
                for s, (hh, pc) in enumerate(((h0, pctx0), (h1, pctx1))):
                    nc.vector.tensor_copy(
                        ctx_sb[s * 64:s * 64 + DK,
                               hp * S + c * 512: hp * S + (c + 1) * 512],
                        pc[0:DK, :])
                    nc.scalar.copy(
                        sums_sb[hh * 32:hh * 32 + 1, c * 512:(c + 1) * 512],
                        pc[DK:DK + 1, :])
            # normalization of chunk c (all 4 heads): broadcast recip row
            # h*32 to all 128 partitions via a one-hot-stationary PE matmul
            # (reuses the o_proj psum banks; fast, no gpsimd chain latency)
            csl = slice(c * 512, (c + 1) * 512)
            nc.vector.reciprocal_approx_fast(rcp_sb[:, csl], sums_sb[:, csl])
            nc.scalar.copy(rcpb_sb[:, csl], rcp_sb[:, csl])
            for h in range(HPC):
                r0 = (h % 2) * 64
                pb = opsp.tile([128, 512], F32, tag=f"po{h % 2}")
                nc.tensor.matmul(pb[:], ehm_sb[:, h * 128:(h + 1) * 128],
                                 rcpb_sb[:, csl], start=True, stop=True,
                                 skip_group_check=True)
                bc = nsb.tile([128, 512], BF16, tag="bc")
                nc.vector.tensor_copy(bc[:], pb[:])
                dst = ctx_sb[r0:r0 + DK,
                             (h // 2) * S + c * 512:(h // 2) * S + (c + 1) * 512]
                nc.vector.tensor_mul(dst, dst, bc[r0:r0 + DK, :])
            # o_proj deferred one chunk: its matmuls land after the NEXT
            # chunk's attention in the PE stream, filling the norm-latency gap
            if prev_c is not None:
                o_proj(prev_c, opsp, obp)
            prev_c = c
        o_proj(prev_c, opsp, obp)
    _stack.close()


def build_nc():
    nc = bacc.Bacc("TRN2", target_bir_lowering=False, debug=False,
                   enable_asserts=False, num_devices=NCORES)
    xt = nc.dram_tensor("xt", [D, S], BF16, kind="ExternalInput").ap()
    wq = nc.dram_tensor("wq", [D, CW], BF16, kind="ExternalInput").ap()
    wk = nc.dram_tensor("wk", [D, CW], BF16, kind="ExternalInput").ap()
    wv = nc.dram_tensor("wv", [D, CW], BF16, kind="ExternalInput").ap()
    wo = nc.dram_tensor("wo", [CW, D], BF16, kind="ExternalInput").ap()
    cs = nc.dram_tensor("cs", [128, S], BF16, kind="ExternalInput").ap()
    sn = nc.dram_tensor("sn", [128, S], BF16, kind="ExternalInput").ap()
    mk = nc.dram_tensor("mk", [128, 1024], BF16, kind="ExternalInput").ap()
    ehm = nc.dram_tensor("ehm", [128, HPC * 128], BF16, kind="ExternalInput").ap()
    pm = nc.dram_tensor("pm", [128, 128], BF16, kind="ExternalInput").ap()
    out = nc.dram_tensor("out_partial", [S, D], BF16, kind="ExternalOutput").ap()
    with tile.TileContext(nc) as tc:
        _build_kernel(tc, nc, xt, wq, wk, wv, wo, cs, sn, mk, pm, ehm, out)
    nc.compile()
    return nc


def _bf16(a):
    """Round-to-nearest-even f32 -> bfloat16 via bit tricks (fast, no ml_dtypes
    conversion loops)."""
    import ml_dtypes
    a = np.ascontiguousarray(a, dtype=np.float32)
    u = a.view(np.uint32)
    r = ((u >> 16) & 1) + np.uint32(0x7FFF)
    return ((u + r) >> 16).astype(np.uint16).view(ml_dtypes.bfloat16)


def make_in_maps(in_features, q_proj_weight, k_proj_weight, v_proj_weight,
                 o_proj_weight, token_positions):
    x = np.asarray(in_features, dtype=np.float32)
    wq = np.asarray(q_proj_weight, dtype=np.float32)
    wk = np.asarray(k_proj_weight, dtype=np.float32)
    wv = np.asarray(v_proj_weight, dtype=np.float32)
    wo = np.asarray(o_proj_weight, dtype=np.float32)
    pos = np.asarray(token_positions).astype(np.float64)

    inv = ROPE_THETA ** (-2.0 * np.arange(DK // 2, dtype=np.float64) / DK)
    ang = inv[:, None] * pos[None, :]  # [32, S]
    c32, s32 = np.cos(ang), np.sin(ang)
    # rows: per 64-row head block, [even(32); odd(32)]; repeats for 2 heads
    cs_full = _bf16(np.tile(c32, (4, 1)))
    sn_full = _bf16(np.concatenate([-s32, s32, -s32, s32], axis=0))

    p = np.arange(128)[:, None]
    f = np.arange(512)[None, :]
    # one triangle, duplicated for the two head streams of a psum pair
    mk = _bf16(np.tile((f >= p).astype(np.float32), (1, 2)))

    pm = _bf16(np.equal(np.arange(128)[:, None] ^ 32,
                        np.arange(128)[None, :]).astype(np.float32))
    ehm_f = np.zeros((128, HPC * 128), np.float32)
    for h in range(HPC):
        ehm_f[h * 32, h * 128:(h + 1) * 128] = 1.0
    ehm = _bf16(ehm_f)

    xb = [np.ascontiguousarray(_bf16(x[b]).T) for b in range(B)]
    in_maps = []
    wq_c = {}
    for c in range(NCORES):
        b, g = c // 4, c % 4
        if g not in wq_c:
            cols = np.arange(g * CW, (g + 1) * CW)
            hcols = cols.reshape(HPC, DK)
            qcols = np.concatenate([np.concatenate([hcols[h, 0::2],
                                                    hcols[h, 1::2]])
                                    for h in range(HPC)])
            wq_c[g] = (
                np.ascontiguousarray(_bf16(wq[qcols, :]).T),
                np.ascontiguousarray(_bf16(wk[qcols, :]).T),
                np.ascontiguousarray(_bf16(wv[cols, :]).T),
                np.ascontiguousarray(_bf16(wo[:, cols]).T),
            )
        wq_g, wk_g, wv_g, wo_g = wq_c[g]
        in_maps.append({
            "xt": xb[b],
            "wq": wq_g,
            "wk": wk_g,
            "wv": wv_g,
            "wo": wo_g,
            "cs": cs_full,
            "sn": sn_full,
            "mk": mk,
            "pm": pm,
            "ehm": ehm,
        })
    return in_maps


_NC_CACHE = []
last_exec_ns = None


def kernel(in_features, q_proj_weight, k_proj_weight, v_proj_weight,
           o_proj_weight, token_positions, d_model=1024, num_heads=16,
           **_ignored):
    global last_exec_ns
    assert int(d_model) == D and int(num_heads) == H
    in_maps = make_in_maps(in_features, q_proj_weight, k_proj_weight,
                           v_proj_weight, o_proj_weight, token_positions)
    if not _NC_CACHE:
        _NC_CACHE.append(build_nc())
    nc = _NC_CACHE[0]
    trace = bool(int(os.environ.get("KERNEL_TRACE", "0")))
    res = bass_utils.run_bass_kernel_spmd(nc, in_maps,
                                          core_ids=list(range(NCORES)),
                                          trace=trace)
    last_exec_ns = res.exec_time_ns
    parts = [np.asarray(r["out_partial"]).astype(np.float32)
             for r in res.results]
    out = np.stack([parts[0] + parts[1] + parts[2] + parts[3],
                    parts[4] + parts[5] + parts[6] + parts[7]])
    return out


# revision 46
# speedup vs baseline: 1.0151x; 1.0151x over previous
"""Trainium2 Bass kernel: single-layer causal attention block (q/k/v/o + RoPE).

Sharding: 8 cores = 2 batches x 4 head-groups (4 heads each).
Per core (SPMD, differs only in input data), all matmul operands bf16:
  - q/k projections emit a merged per-head [even(32); odd(32)] row layout so
    each score tile is ONE 64-contraction matmul (vs 2x32 split).
  - RoPE: partner rows fetched via a PE permutation matmul (row r <- r^32),
    then dst = A*CS + rot*SN in 3 full-width DVE ops per 512-chunk.
  - scores^T = K-stationary matmul -> exp (bf16) on ACT -> multiplicative
    causal mask on diagonal tiles -> PV with a ones-column rowsum.
  - softmax normalization batched: rowsums -> one reciprocal_approx_fast,
    per-(head,chunk) broadcast via one-hot-stationary PE matmul.
  - o_proj partial [2048, 1024] emitted in bf16.
Host: sums the 4 per-head-group partials per batch (row-sharded o_proj
unshard) in f32 and stacks the 2 batches.
"""

import os
import sys

import numpy as np

sys.path.insert(0, "/opt/trn_rl_repo")

import concourse.bass as bass  # noqa: E402
import concourse.tile as tile  # noqa: E402
from concourse import bacc, mybir  # noqa: E402
from concourse import bass_utils  # noqa: E402

B, S, D, H, DK = 2, 2048, 1024, 16, 64
NCORES = 8
HPC = H // 4  # 4 heads per core
CW = HPC * DK  # 256 head-dim columns per core
VW = DK + 1  # 65: v width per head incl ones column
ND = D // 128  # 8 contraction chunks
NS = S // 128  # 16 s-tiles
NSC = S // 512  # 4 s-chunks
ROPE_THETA = 10000.0

F32 = mybir.dt.float32
BF16 = mybir.dt.bfloat16
EXP = mybir.ActivationFunctionType.Exp


def _build_kernel(tc, nc, xt, wq, wk, wv, wo, cs, sn, mk, pm, out):
    from contextlib import ExitStack
    _stack = ExitStack()
    constp = _stack.enter_context(tc.tile_pool(name="const", bufs=1))
    pers = _stack.enter_context(tc.tile_pool(name="persist", bufs=1))
    xtp = _stack.enter_context(tc.tile_pool(name="xt", bufs=1))

    wq_sb = constp.tile([128, ND * CW], BF16)
    wk_sb = constp.tile([128, ND * CW], BF16)
    wv_sb = constp.tile([128, ND * CW], BF16)
    wo_sb = constp.tile([128, 2 * D], BF16)
    cs_sb = constp.tile([128, S], BF16)
    sn_sb = constp.tile([128, S], BF16)
    mk_sb = constp.tile([128, 1024], BF16)
    pm_sb = constp.tile([128, 128], BF16)
    xts = xtp.tile([128, ND * S], BF16)
    # DMA order tuned for time-to-first-matmul: weights and the first half of
    # x^T land first; attention/o_proj-only tables go last. Weight tensors are
    # single DMA instructions (rearranged APs) to cut sync-queue issue time.
    xts3 = xts[:].rearrange("p (d s) -> p d s", s=S)
    wq3 = wq[:].rearrange("(d p) c -> p d c", p=128)
    wk3 = wk[:].rearrange("(d p) c -> p d c", p=128)
    wq_sb3 = wq_sb[:].rearrange("p (d c) -> p d c", c=CW)
    wk_sb3 = wk_sb[:].rearrange("p (d c) -> p d c", c=CW)
    # two hwdge queues (sync + scalar) stream the prologue in parallel
    xt3 = xt[:].rearrange("(d p) s -> p d s", p=128)
    nc.sync.dma_start(wq_sb3[:, 0:4], wq3[:, 0:4])
    nc.scalar.dma_start(wq_sb3[:, 4:8], wq3[:, 4:8])
    nc.sync.dma_start(xts3[:, 0:4, 0:512], xt3[:, 0:4, 0:512])
    nc.scalar.dma_start(xts3[:, 4:8, 0:512], xt3[:, 4:8, 0:512])
    nc.sync.dma_start(xts3[:, 0:4, 512:1024], xt3[:, 0:4, 512:1024])
    nc.scalar.dma_start(xts3[:, 4:8, 512:1024], xt3[:, 4:8, 512:1024])
    nc.sync.dma_start(wk_sb3[:, 0:4], wk3[:, 0:4])
    nc.scalar.dma_start(wk_sb3[:, 4:8], wk3[:, 4:8])
    nc.sync.dma_start(cs_sb[:], cs[:])
    nc.scalar.dma_start(sn_sb[:], sn[:])
    nc.sync.dma_start(pm_sb[:], pm[:])
    nc.scalar.dma_start(wv_sb[:].rearrange("p (d c) -> p d c", c=CW),
                        wv[:].rearrange("(d p) c -> p d c", p=128))
    nc.sync.dma_start(xts3[:, 0:4, 1024:S], xt3[:, 0:4, 1024:S])
    nc.scalar.dma_start(xts3[:, 4:8, 1024:S], xt3[:, 4:8, 1024:S])
    nc.sync.dma_start(mk_sb[:], mk[:])
    nc.sync.dma_start(wo_sb[:].rearrange("p (b c) -> p b c", c=D),
                      wo[:].rearrange("(b p) c -> p b c", p=128))

    # q/k: one [64, S] tile per head, rows = [even(32); odd(32)], so every
    # score matmul runs at tile_position (0,0) (no PE retile between score
    # and PV matmuls).
    qh = [pers.tile([64, S], BF16, name=f"qh{_h}") for _h in range(HPC)]
    kh = [pers.tile([64, S], BF16, name=f"kh{_h}") for _h in range(HPC)]
    v_sb = pers.tile([128, NS * HPC * VW], BF16)
    ctx_sb = pers.tile([128, 2 * S], BF16)
    # rowsum rows live at partition h*32 (engine partition starts must be
    # multiples of 32); unused rows memset to 1.0 so reciprocal stays finite
    sums_sb = pers.tile([128, S], F32)
    rcp_sb = pers.tile([128, S], F32)
    rcpb_sb = pers.tile([128, S], BF16)

    v3 = v_sb[:].rearrange("p (t c) -> p t c", c=VW)
    nc.vector.memset(v3[:, :, DK:DK + 1], 1.0)
    nc.vector.memset(sums_sb[:], 1.0)

    # ---- Phase 1: projections + RoPE ----
    # Per sc-chunk: a single-bank accumulation chain over d (consecutive PE
    # matmuls hit the same PSUM bank - the fast PE issue path), then RoPE.
    with tc.tile_pool(name="pp", bufs=3, space="PSUM") as pp, \
         tc.tile_pool(name="rp", bufs=2, space="PSUM") as rp, \
         tc.tile_pool(name="vps", bufs=2, space="PSUM") as vps, \
         tc.tile_pool(name="rsb", bufs=8) as rsb:
        blocks = [(qh[0], qh[1], wq_sb, 0), (qh[2], qh[3], wq_sb, 1),
                  (kh[0], kh[1], wk_sb, 0), (kh[2], kh[3], wk_sb, 1)]
        for dst0, dst1, wsb, blk in blocks:
            for sc in range(NSC):
                pst = pp.tile([128, 512], F32, tag="pjt")
                for d in range(ND):
                    nc.tensor.matmul(
                        pst[:],
                        wsb[:, d * CW + blk * 128: d * CW + blk * 128 + 128],
                        xts[:, d * S + sc * 512: d * S + (sc + 1) * 512],
                        start=(d == 0), stop=(d == ND - 1))
                araw = rsb.tile([128, 512], BF16, tag="araw")
                nc.vector.tensor_copy(araw[:], pst[:])
                prot = rp.tile([128, 512], F32)
                nc.tensor.matmul(prot[:], pm_sb[:], araw[:], start=True, stop=True)
                rot = rsb.tile([128, 512], BF16, tag="rot")
                nc.scalar.copy(rot[:], prot[:])
                t1 = rsb.tile([128, 512], BF16, tag="t1")
                t2 = rsb.tile([128, 512], BF16, tag="t2")
                nc.vector.tensor_mul(t1[:], araw[:], cs_sb[:, sc * 512:(sc + 1) * 512])
                nc.vector.tensor_mul(t2[:], rot[:], sn_sb[:, sc * 512:(sc + 1) * 512])
                nc.vector.tensor_add(dst0[:, sc * 512:(sc + 1) * 512],
                                     t1[0:64, :], t2[0:64, :])
                nc.vector.tensor_add(dst1[:, sc * 512:(sc + 1) * 512],
                                     t1[64:128, :], t2[64:128, :])
        for sm in range(NS):
            pv = vps.tile([128, CW], F32)
            for d in range(ND):
                nc.tensor.matmul(
                    pv[:],
                    xts[:, d * S + sm * 128: d * S + sm * 128 + 128],
                    wv_sb[:, d * CW:(d + 1) * CW],
                    start=(d == 0), stop=(d == ND - 1))
            base = sm * HPC * VW
            dst3 = v_sb[:, base:base + HPC * VW].rearrange("p (h c) -> p h c", c=VW)
            nc.vector.tensor_copy(dst3[:, :, 0:DK],
                                  pv[:].rearrange("p (h c) -> p h c", c=DK))

    # ---- Phase 2: attention + per-chunk normalization + o_proj ----
    # Heads (2hp, 2hp+1) share one [128,1024] psum pair-tile per k-tile: two
    # score matmuls, ONE exp over the pair, two PV accumulations. After both
    # head-pairs finish a q-chunk, normalize it (gpsimd broadcast of the
    # batched reciprocal) and emit its o_proj s-tiles, overlapping the next
    # chunk's attention.
    def o_proj(c, opsp, obp):
        for sm in range(4 * c, 4 * c + 4):
            pos = [opsp.tile([128, 512], F32, name=f"po{_i}", tag=f"po{_i}")
                   for _i in range(2)]
            for do_ in range(2):
                for cb in range(2):
                    nc.tensor.matmul(
                        pos[do_][:],
                        ctx_sb[:, cb * S + sm * 128: cb * S + sm * 128 + 128],
                        wo_sb[:, cb * D + do_ * 512: cb * D + (do_ + 1) * 512],
                        start=(cb == 0), stop=(cb == 1),
                        skip_group_check=True)
            ot = obp.tile([128, 1024], BF16)
            nc.vector.tensor_copy(ot[:, 0:512], pos[0][:])
            nc.scalar.copy(ot[:, 512:1024], pos[1][:])
            nc.sync.dma_start(out[sm * 128:(sm + 1) * 128, :], ot[:])

    with tc.tile_pool(name="sps", bufs=2, space="PSUM") as sps, \
         tc.tile_pool(name="cps", bufs=1, space="PSUM") as cps, \
         tc.tile_pool(name="ops", bufs=1, space="PSUM") as opsp, \
         tc.tile_pool(name="expool", bufs=10) as exp_pool, \
         tc.tile_pool(name="nsb", bufs=4) as nsb, \
         tc.tile_pool(name="obuf", bufs=2) as obp:
        prev_c = None
        for c in (3, 2, 1, 0):
            nsk = 4 * (c + 1)
            for hp in range(2):
                h0, h1 = 2 * hp, 2 * hp + 1
                pctx0 = cps.tile([VW, 512], F32, tag="pctx0")
                pctx1 = cps.tile([VW, 512], F32, tag="pctx1")
                exps = []

                def pv_pair(ta, tb, nsk=nsk, pctx0=pctx0, pctx1=pctx1,
                            exps=exps, h0=h0, h1=h1, c=c):
                    # two consecutive PV accumulations per pctx bank
                    # back-to-back: same-bank chains issue fastest on the PE
                    for s, (hh, pc) in enumerate(((h0, pctx0), (h1, pctx1))):
                        for t in (ta, tb):
                            qo = max(0, t - 4 * c) * 128
                            vbase = t * HPC * VW + hh * VW
                            nc.tensor.matmul(
                                pc[:, qo:512],
                                v_sb[:, vbase:vbase + VW],
                                exps[t][:, s * 512: s * 512 + 512 - qo],
                                start=(t == 0), stop=(t == nsk - 1),
                                skip_group_check=True)

                for pi in range(nsk // 2):
                    for t in (2 * pi, 2 * pi + 1):
                        j = t - 4 * c
                        qo = max(0, j) * 128
                        w = 512 - qo
                        pscore = sps.tile([128, 1024], F32)
                        for s, hh in enumerate((h0, h1)):
                            nc.tensor.matmul(
                                pscore[:, s * 512: s * 512 + w],
                                kh[hh][:, t * 128:(t + 1) * 128],
                                qh[hh][:, c * 512 + qo:(c + 1) * 512],
                                start=True, stop=True,
                                skip_group_check=True)
                        et = exp_pool.tile([128, 1024], BF16)
                        if j <= 0:
                            nc.scalar.activation(et[:], pscore[:], EXP,
                                                 scale=0.125)
                            if j == 0:
                                nc.vector.tensor_mul(et[:], et[:], mk_sb[:])
                        else:
                            # shifted coords: every diagonal mask is the same
                            # triangle (f' >= p), truncated to width w
                            for s in range(2):
                                nc.scalar.activation(
                                    et[:, s * 512:s * 512 + w],
                                    pscore[:, s * 512:s * 512 + w],
                                    EXP, scale=0.125)
                                nc.vector.tensor_mul(
                                    et[:, s * 512:s * 512 + w],
                                    et[:, s * 512:s * 512 + w],
                                    mk_sb[:, 0:w])
                        exps.append(et)
                    if pi >= 3:
                        pv_pair(2 * (pi - 3), 2 * pi - 5)
                for pi in range(max(0, nsk // 2 - 3), nsk // 2):
                    pv_pair(2 * pi, 2 * pi + 1)
                # ctx on DVE, rowsums on ACT: both streams drain in parallel
                for s, (hh, pc) in enumerate(((h0, pctx0), (h1, pctx1))):
                    nc.vector.tensor_copy(
                        ctx_sb[s * 64:s * 64 + DK,
                               hp * S + c * 512: hp * S + (c + 1) * 512],
                        pc[0:DK, :])
                    nc.scalar.copy(
                        sums_sb[hh * 32:hh * 32 + 1, c * 512:(c + 1) * 512],
                        pc[DK:DK + 1, :])
            # normalization of chunk c (all 4 heads)
            csl = slice(c * 512, (c + 1) * 512)
            nc.vector.reciprocal_approx_fast(rcp_sb[:, csl], sums_sb[:, csl])
            for h in range(HPC):
                r0 = (h % 2) * 64
                # cast+stage the recip row at partition 0 in one ACT op: hw
                # partition_broadcast reads the tile's partition 0 regardless
                # of the AP offset
                rc1 = nsb.tile([1, 512], BF16, tag="rc1")
                nc.scalar.copy(rc1[:], rcp_sb[h * 32:h * 32 + 1, csl])
                bc = nsb.tile([128, 512], BF16, tag="bc")
                nc.gpsimd.partition_broadcast(bc[:], rc1[:])
                dst = ctx_sb[r0:r0 + DK,
                             (h // 2) * S + c * 512:(h // 2) * S + (c + 1) * 512]
                nc.vector.tensor_mul(dst, dst, bc[r0:r0 + DK, :])
            # o_proj deferred one chunk: its matmuls land after the NEXT
            # chunk's attention in the PE stream, filling the norm-latency gap
            if prev_c is not None:
                o_proj(prev_c, opsp, obp)
            prev_c = c
        o_proj(prev_c, opsp, obp)
    _stack.close()


def build_nc():
    nc = bacc.Bacc("TRN2", target_bir_lowering=False, debug=False,
                   enable_asserts=False, num_devices=NCORES)
    xt = nc.dram_tensor("xt", [D, S], BF16, kind="ExternalInput").ap()
    wq = nc.dram_tensor("wq", [D, CW], BF16, kind="ExternalInput").ap()
    wk = nc.dram_tensor("wk", [D, CW], BF16, kind="ExternalInput").ap()
    wv = nc.dram_tensor("wv", [D, CW], BF16, kind="ExternalInput").ap()
    wo = nc.dram_tensor("wo", [CW, D], BF16, kind="ExternalInput").ap()
    cs = nc.dram_tensor("cs", [128, S], BF16, kind="ExternalInput").ap()
    sn = nc.dram_tensor("sn", [128, S], BF16, kind="ExternalInput").ap()
    mk = nc.dram_tensor("mk", [128, 1024], BF16, kind="ExternalInput").ap()
    pm = nc.dram_tensor("pm", [128, 128], BF16, kind="ExternalInput").ap()
    out = nc.dram_tensor("out_partial", [S, D], BF16, kind="ExternalOutput").ap()
    with tile.TileContext(nc) as tc:
        _build_kernel(tc, nc, xt, wq, wk, wv, wo, cs, sn, mk, pm, out)
    nc.compile()
    return nc


def _bf16(a):
    """Round-to-nearest-even f32 -> bfloat16 via bit tricks (fast, no ml_dtypes
    conversion loops)."""
    import ml_dtypes
    a = np.ascontiguousarray(a, dtype=np.float32)
    u = a.view(np.uint32)
    r = ((u >> 16) & 1) + np.uint32(0x7FFF)
    return ((u + r) >> 16).astype(np.uint16).view(ml_dtypes.bfloat16)


def make_in_maps(in_features, q_proj_weight, k_proj_weight, v_proj_weight,
                 o_proj_weight, token_positions):
    x = np.asarray(in_features, dtype=np.float32)
    wq = np.asarray(q_proj_weight, dtype=np.float32)
    wk = np.asarray(k_proj_weight, dtype=np.float32)
    wv = np.asarray(v_proj_weight, dtype=np.float32)
    wo = np.asarray(o_proj_weight, dtype=np.float32)
    pos = np.asarray(token_positions).astype(np.float64)

    inv = ROPE_THETA ** (-2.0 * np.arange(DK // 2, dtype=np.float64) / DK)
    ang = inv[:, None] * pos[None, :]  # [32, S]
    c32, s32 = np.cos(ang), np.sin(ang)
    # rows: per 64-row head block, [even(32); odd(32)]; repeats for 2 heads
    cs_full = _bf16(np.tile(c32, (4, 1)))
    sn_full = _bf16(np.concatenate([-s32, s32, -s32, s32], axis=0))

    p = np.arange(128)[:, None]
    f = np.arange(512)[None, :]
    # one triangle, duplicated for the two head streams of a psum pair
    mk = _bf16(np.tile((f >= p).astype(np.float32), (1, 2)))

    pm = _bf16(np.equal(np.arange(128)[:, None] ^ 32,
                        np.arange(128)[None, :]).astype(np.float32))

    xb = [np.ascontiguousarray(_bf16(x[b]).T) for b in range(B)]
    in_maps = []
    wq_c = {}
    for c in range(NCORES):
        b, g = c // 4, c % 4
        if g not in wq_c:
            cols = np.arange(g * CW, (g + 1) * CW)
            hcols = cols.reshape(HPC, DK)
            qcols = np.concatenate([np.concatenate([hcols[h, 0::2],
                                                    hcols[h, 1::2]])
                                    for h in range(HPC)])
            wq_c[g] = (
                np.ascontiguousarray(_bf16(wq[qcols, :]).T),
                np.ascontiguousarray(_bf16(wk[qcols, :]).T),
                np.ascontiguousarray(_bf16(wv[cols, :]).T),
                np.ascontiguousarray(_bf16(wo[:, cols]).T),
            )
        wq_g, wk_g, wv_g, wo_g = wq_c[g]
        in_maps.append({
            "xt": xb[b],
            "wq": wq_g,
            "wk": wk_g,
            "wv": wv_g,
            "wo": wo_g,
            "cs": cs_full,
            "sn": sn_full,
            "mk": mk,
            "pm": pm,
        })
    return in_maps


_NC_CACHE = []
last_exec_ns = None


def kernel(in_features, q_proj_weight, k_proj_weight, v_proj_weight,
           o_proj_weight, token_positions, d_model=1024, num_heads=16,
           **_ignored):
    global last_exec_ns
    assert int(d_model) == D and int(num_heads) == H
    in_maps = make_in_maps(in_features, q_proj_weight, k_proj_weight,
                           v_proj_weight, o_proj_weight, token_positions)
    if not _NC_CACHE:
        _NC_CACHE.append(build_nc())
    nc = _NC_CACHE[0]
    trace = bool(int(os.environ.get("KERNEL_TRACE", "0")))
    res = bass_utils.run_bass_kernel_spmd(nc, in_maps,
                                          core_ids=list(range(NCORES)),
                                          trace=trace)
    last_exec_ns = res.exec_time_ns
    parts = [np.asarray(r["out_partial"]).astype(np.float32)
             for r in res.results]
    out = np.stack([parts[0] + parts[1] + parts[2] + parts[3],
                    parts[4] + parts[5] + parts[6] + parts[7]])
    return out
